# revision 36
# baseline (speedup 1.0000x reference)
"""DiffAttn3d Trainium2 kernel (v6.1: v6 + single-Newton fast-invsqrt).

8-core sharding: core c -> (batch b = c//4, query slice qs = (c%4)*512).
Each core computes its 512-query slice of the full differential-attention
block (all 16 n-heads) and the final output projection for that slice.

Structure:
- Host pre-transposes x (xsT/xqT bf16) and pre-packs all weights in bf16.
- Heads packed 4-per-128-partitions (offsets 0/32/64/96); the two QK^T
  matmuls of a head pair run concurrently on their two 32-row PE groups
  (explicit tile_position row tiling).
- Software-pipelined by head pair: phase i streams QK -> exp ([128,1024]
  ACT batches, double-buffered 2-bank score PSUM) -> multiplicative bf16
  mask (DVE broadcast AP) into pair i's SBUF e-buffer, while the PE queue
  interleaves the AV matmuls of pair i-1 reading the previous e-buffer.
  Engine queues are strict FIFO; this emission order is the schedule.
- AV accumulates [80, 512] per n-head in PSUM (ones columns 64:80 give
  the softmax denominator; 16 of them so the xbar 16-row tile covers the
  denominator in the same transpose).
- Epilogue per pair: PSUM->SBUF bf16 copies, then ONE 3-D xbar DMA
  transpose per tensor ([80,512] -> [128,4,80]) back to natural [q, d]
  layout, batched DVE combine + fast inverse sqrt on [128,4]-shaped
  scalars (partition-parallel), one DMA transpose of the scaled attn
  rows ([128,4,128]; junk cols never read), and the output projection
  accumulated across all 8 pairs x 4 subtiles in one persistent PSUM
  bank (single accumulation group - start clears has_written bank-wide).
PSUM: 4 (scores x2) + 2 (av) + 1 (out proj) = 7 of 8 banks.

The fast inverse sqrt uses ONE Newton iteration (max ~0.2% error on the
RMS-norm scale, far inside the 2e-2 budget); the second iteration's 4
DVE ops x 8 pairs sat on the busiest engine. Measured 190464 ns vs
192755 ns with two iterations (rel err 5.01e-3 vs 4.91e-3).

Alternatives measured SLOWER than this three-way balance (PE ~138us /
ACT ~138us / DVE ~144us active, ~73% pipeline coupling efficiency):
- Folding the mask additively into score PSUM via identity matmuls
  (PE + ~450ns/unit of serial full-array matmuls -> 204-214us).
- fp8e4 exp/V with DoubleRow AV: DR gave only ~5% over bf16 on the
  streaming side (404ns vs 426ns per unit) while fp8-e alone costs
  ~1.4e-2 rel err. Not worth it.
- Schraudolph bit-trick exp on DVE (tensor_scalar f32->int16, RNE
  saturating, bits = 184.665*s + 16250.9): works and is accurate to
  ~3%/element, but DVE has no spare capacity outside phase 0.
- Routing deferred projections through the idle out-proj PSUM bank:
  cross-engine waits inside the PE FIFO stall everything (241us).
"""

import math
import numpy as np

B, L, IN_DIM, OUT_DIM = 2, 2048, 128, 128
H, DH = 8, 32
ED = H * DH * 2          # 512
NH = 2 * H               # 16 n-heads
DEPTH = 1
LAMBDA_INIT = 0.8 - 0.6 * math.exp(-0.3 * (DEPTH + 1))
EPS = 1e-8

QSL = 512                # queries per core
NKC = L // 128           # 16 key chunks
NQS = QSL // 128         # 4 query subtiles
NCH = 4                  # head chunks: 4 heads per 128 partitions
AVP = 80                 # av rows: 64 v-dims + 16 ones (xbar tile = 16 rows)

_CACHE = {}


def _build_program():
    import concourse.bass as bass
    import concourse.tile as tile
    from concourse import bacc, mybir

    f32 = mybir.dt.float32
    bf16 = mybir.dt.bfloat16
    u32 = mybir.dt.uint32
    AF = mybir.ActivationFunctionType
    ALU = mybir.AluOpType

    nc = bacc.Bacc("TRN2", target_bir_lowering=False, debug=False,
                   num_devices=8)

    xsT_d = nc.declare_dram_parameter("xsT", [IN_DIM, L], bf16, isOutput=False)
    xqT_d = nc.declare_dram_parameter("xqT", [IN_DIM, QSL], bf16,
                                      isOutput=False)
    mT_d = nc.declare_dram_parameter("maskT", [L, QSL], bf16, isOutput=False)
    # w = [Wq4 | Wk4 | Wv]: Wq/Wk packed 4 heads per 128 cols (offsets
    # 0/32/64/96), Wq pre-scaled by DH^-0.5.
    w_d = nc.declare_dram_parameter("w", [IN_DIM, 3 * ED], bf16,
                                    isOutput=False)
    wo_d = nc.declare_dram_parameter("wo", [64, H, OUT_DIM], bf16,
                                     isOutput=False)
    nlam_d = nc.declare_dram_parameter("nlam", [128, 1], f32, isOutput=False)
    out_d = nc.declare_dram_parameter("out", [QSL, OUT_DIM], f32, isOutput=True)

    with tile.TileContext(nc) as tc:
        with (
            tc.tile_pool(name="const", bufs=1) as const,
            tc.tile_pool(name="psA", bufs=2, space=bass.MemorySpace.PSUM) as psA,
            tc.tile_pool(name="avp", bufs=1, space=bass.MemorySpace.PSUM) as avp,
            tc.tile_pool(name="outp", bufs=1, space=bass.MemorySpace.PSUM) as outp,
            tc.tile_pool(name="epool", bufs=2) as epool,
            tc.tile_pool(name="natp", bufs=2) as natp,
            tc.tile_pool(name="tinyp", bufs=4) as tinyp,
            tc.tile_pool(name="tmpp", bufs=3) as tmpp,
        ):
            # ---- constants / weights (all DMA'd pre-packed in bf16) ----
            w_sb = const.tile([128, 3 * ED], bf16)
            nc.sync.dma_start(w_sb[:], w_d[:])
            wo_sb = const.tile([64, H, OUT_DIM], bf16)
            nc.sync.dma_start(wo_sb[:], wo_d[:])
            nlam_sb = const.tile([128, 1], f32)
            nc.sync.dma_start(nlam_sb[:], nlam_d[:])
            xsT = const.tile([128, L], bf16)
            nc.sync.dma_start(xsT[:], xsT_d[:])
            xqT = const.tile([128, QSL], bf16)
            nc.sync.dma_start(xqT[:], xqT_d[:])
            mkf = const.tile([128, NKC, QSL], bf16)
            nc.sync.dma_start(mkf[:], mT_d.rearrange("(kc p) q -> p kc q",
                                                     p=128))
            magic = const.tile([128, NQS], u32)
            nc.vector.memset(magic[:], 0x5F3759DF)

            # ---- projections (bf16 in/out, fp32 psum) ----
            # Only chunk 0's qT/kT are emitted up front (the minimum to
            # start the exp stream); everything else is deferred into the
            # early phases' kc slots via setup_items.
            qT = const.tile([128, NCH, QSL], bf16)
            kT = const.tile([128, NCH, L], bf16)
            vp = const.tile([128, NKC, H, AVP], bf16)
            nc.vector.memset(vp[:, :, :, 64:AVP], 1.0)

            def proj_q(c):
                def emit():
                    ps = psA.tile([128, 2, QSL], f32, tag="sps", name="ps")
                    nc.tensor.matmul(ps[:, 0, :],
                                     w_sb[:, c * 128:(c + 1) * 128],
                                     xqT[:], start=True, stop=True)
                    nc.vector.tensor_copy(qT[:, c, :], ps[:, 0, :])
                return emit

            def proj_k(c, h2):
                def emit():
                    ps = psA.tile([128, 2, QSL], f32, tag="sps", name="ps")
                    for s in range(2):
                        nc.tensor.matmul(
                            ps[:, s, :],
                            w_sb[:, ED + c * 128:ED + (c + 1) * 128],
                            xsT[:, (h2 * 2 + s) * 512:(h2 * 2 + s + 1) * 512],
                            start=True, stop=True)
                    nc.vector.tensor_copy(
                        kT[:, c, h2 * 1024:(h2 + 1) * 1024],
                        ps[:].rearrange("p s q -> p (s q)"))
                return emit

            def proj_v(st2):
                def emit():
                    ps = psA.tile([128, 2, QSL], f32, tag="sps", name="ps")
                    for s in range(2):
                        st = st2 * 2 + s
                        nc.tensor.matmul(ps[:, s, :],
                                         xsT[:, st * 128:(st + 1) * 128],
                                         w_sb[:, 2 * ED:3 * ED],
                                         start=True, stop=True)
                    for s in range(2):
                        st = st2 * 2 + s
                        nc.vector.tensor_copy(
                            vp[:, st, :, 0:64],
                            ps[:, s, :].rearrange("p (h d) -> p h d", h=H))
                return emit

            for f in (proj_q(0), proj_k(0, 0), proj_k(0, 1)):
                f()
            # deferred: vp during phase 0 (used by AV from phase 1 on),
            # chunk c's qT/kT during phase 2c-2 (used from phase 2c on)
            setup_items = {
                0: [proj_v(s) for s in range(NKC // 2)],
                1: [proj_q(1), proj_k(1, 0), proj_k(1, 1)],
                2: [proj_q(2), proj_k(2, 0), proj_k(2, 1)],
                3: [proj_q(3), proj_k(3, 0), proj_k(3, 1)],
            }

            out_ps = outp.tile([128, NQS, 128], f32, tag="op", name="out_ps")
            n_outmm = [0]

            def pass1(i, kc, eb):
                c, p = divmod(i, 2)
                sps = psA.tile([128, 2, QSL], f32, tag="sps", name="sps")
                for j in range(2):
                    r = 64 * p + 32 * j
                    nc.tensor.matmul(
                        sps[:, j, :],
                        kT[r:r + 32, c, kc * 128:(kc + 1) * 128],
                        qT[r:r + 32, c, :],
                        start=True, stop=True, tile_position=(r, 0))
                nc.scalar.activation(eb[:, kc, :, :], sps[:], AF.Exp)
                mb = mkf[:, kc, :].unsqueeze(1).broadcast_to([128, 2, QSL])
                nc.vector.tensor_tensor(eb[:, kc, :, :], eb[:, kc, :, :],
                                        mb, ALU.mult)

            def av_step(i, kc, eb, av):
                for j in range(2):
                    nc.tensor.matmul(av[j][:], vp[:, kc, i, :],
                                     eb[:, kc, j, :],
                                     start=(kc == 0), stop=(kc == NKC - 1))

            def epilogue(i, av):
                nat = []
                for j in range(2):
                    a_sb = tmpp.tile([AVP, QSL], bf16, tag=f"a_sb{j}")
                    nc.vector.tensor_copy(a_sb[:], av[j][:])
                    nt = natp.tile([128, NQS, AVP], bf16, tag=f"nat{j}",
                                   name=f"nat{j}")
                    nc.sync.dma_start_transpose(nt[:], a_sb[:])
                    nat.append(nt)

                r0v = tinyp.tile([128, NQS, 1], f32, tag="r0v")
                nc.vector.reciprocal(r0v[:], nat[0][:, :, 64:65])
                r1v = tinyp.tile([128, NQS, 1], f32, tag="r1v")
                nc.vector.reciprocal(r1v[:], nat[1][:, :, 64:65])
                r1p = tinyp.tile([128, NQS, 1], f32, tag="r1p")
                nc.vector.tensor_scalar(r1p[:], r1v[:], nlam_sb[:], None,
                                        ALU.mult)

                t0 = tmpp.tile([128, NQS, 64], f32, tag="t0")
                nc.vector.tensor_tensor(
                    t0[:], nat[0][:, :, 0:64],
                    r0v[:].broadcast_to([128, NQS, 64]), ALU.mult)
                t1 = tmpp.tile([128, NQS, 64], f32, tag="t1")
                nc.vector.tensor_tensor(
                    t1[:], nat[1][:, :, 0:64],
                    r1p[:].broadcast_to([128, NQS, 64]), ALU.mult)
                at4 = tmpp.tile([128, NQS, 64], bf16, tag="at4")
                nc.vector.tensor_tensor(at4[:], t0[:], t1[:], ALU.add)
                sq4 = tmpp.tile([128, NQS, 64], f32, tag="sq4")
                nc.vector.tensor_tensor(sq4[:], at4[:], at4[:], ALU.mult)
                ss4 = tinyp.tile([128, NQS], f32, tag="ss4")
                nc.vector.tensor_reduce(ss4[:], sq4[:],
                                        mybir.AxisListType.X, ALU.add)

                # rr4 = 1/sqrt(ss4/64): fast inverse sqrt on DVE
                msx = tinyp.tile([128, NQS], f32, tag="msx")
                nc.vector.tensor_scalar(msx[:], ss4[:], 1.0 / 64, None,
                                        ALU.mult)
                sh = tinyp.tile([128, NQS], u32, tag="sh")
                nc.vector.tensor_scalar(sh[:], msx[:].bitcast(u32), 1,
                                        None, ALU.logical_shift_right)
                rr4 = tinyp.tile([128, NQS], f32, tag="rr4")
                nc.vector.tensor_tensor(rr4[:].bitcast(u32), magic[:],
                                        sh[:], ALU.subtract)
                nwu = tinyp.tile([128, NQS], f32, tag="nwu")
                nww = tinyp.tile([128, NQS], f32, tag="nww")
                for _ in range(1):
                    nc.vector.tensor_tensor(nwu[:], rr4[:], rr4[:], ALU.mult)
                    nc.vector.scalar_tensor_tensor(
                        nwu[:], nwu[:], 0.5, msx[:], ALU.mult, ALU.mult)
                    nc.vector.tensor_scalar(nww[:], nwu[:], -1.0, 1.5,
                                            ALU.mult, ALU.add)
                    nc.vector.tensor_tensor(rr4[:], rr4[:], nww[:], ALU.mult)

                # at_s cols 64:128 are junk; the transposed junk rows are
                # never read (projection lhsT slice [0:64])
                at_s = tmpp.tile([128, NQS, 128], bf16, tag="at_s")
                nc.vector.tensor_tensor(
                    at_s[:, :, 0:64], at4[:],
                    rr4[:].unsqueeze(2).broadcast_to([128, NQS, 64]),
                    ALU.mult)
                atT = natp.tile([128, NQS, 128], bf16, tag="atT",
                                name=f"atT{i}")
                nc.sync.dma_start_transpose(atT[:], at_s[:])
                for q in range(NQS):
                    # single accumulation group for the whole bank: start
                    # clears has_written bank-wide
                    nc.tensor.matmul(out_ps[:, q, :], atT[0:64, q, :],
                                     wo_sb[:, i, :],
                                     start=(n_outmm[0] == 0),
                                     stop=(n_outmm[0] == H * NQS - 1))
                    n_outmm[0] += 1

            # ---- attention: software-pipelined over 8 pairs ----
            # phase i: pass1(i) + AV(i-1); the last pair's AV runs inline
            # in phase 7 (after its own mask) so there is no drain phase.
            ebufs, avbufs = {}, {}
            for i in range(H + 1):
                if i < H:
                    ebufs[i] = epool.tile([128, NKC, 2, QSL], bf16,
                                          tag="eall", name=f"eall{i}")
                    avbufs[i] = [avp.tile([AVP, QSL], f32, tag=f"av{j}",
                                          name=f"av{j}_{i}")
                                 for j in range(2)]
                items = setup_items.get(i, [])
                for kc in range(NKC):
                    if i < H:
                        pass1(i, kc, ebufs[i])
                    if items and kc % 2 == 0 and kc // 2 < len(items):
                        items[kc // 2]()
                    if i >= 1:
                        av_step(i - 1, kc, ebufs[i - 1], avbufs[i - 1])
                if i >= 1:
                    epilogue(i - 1, avbufs[i - 1])
                    del ebufs[i - 1], avbufs[i - 1]

            out_sb = const.tile([128, NQS, 128], f32)
            nc.vector.tensor_copy(out_sb[:], out_ps[:])
            nc.sync.dma_start(out_d.rearrange("(s p) o -> p s o", p=128),
                              out_sb[:])

    nc.compile()
    return nc


def kernel(**inputs):
    import ml_dtypes
    from concourse.bass_utils import run_bass_kernel_spmd

    bfdt = ml_dtypes.bfloat16

    x = np.asarray(inputs["x"], np.float32)
    mask = np.asarray(inputs["mask_2d"])
    Wq = np.asarray(inputs["Wq"], np.float32)
    Wkv = np.asarray(inputs["Wkv"], np.float32)
    Wout = np.asarray(inputs["Wout"], np.float32)
    lq1 = np.asarray(inputs["lambda_q1"], np.float32)
    lk1 = np.asarray(inputs["lambda_k1"], np.float32)
    lq2 = np.asarray(inputs["lambda_q2"], np.float32)
    lk2 = np.asarray(inputs["lambda_k2"], np.float32)
    gamma = np.asarray(inputs["gamma"], np.float32)

    lam = float(np.exp(np.sum(lq1 * lk1)) - np.exp(np.sum(lq2 * lk2))
                + LAMBDA_INIT)
    Wq_s = (Wq * DH ** -0.5).astype(np.float32)
    Wk = Wkv[:, :ED]
    Wv = Wkv[:, ED:]

    def pack_heads4(Wm):
        # chunk c (128 cols) holds heads 4c..4c+3 at col offsets 0/32/64/96
        out = np.empty((IN_DIM, NCH * 128), np.float32)
        for n in range(NH):
            c, r = divmod(n, 4)
            out[:, c * 128 + r * 32:c * 128 + r * 32 + 32] = \
                Wm[:, n * DH:(n + 1) * DH]
        return out

    W = np.concatenate([pack_heads4(Wq_s), pack_heads4(Wk), Wv],
                       axis=1).astype(bfdt)
    gs = (gamma * (1.0 - LAMBDA_INIT)).astype(np.float32)
    Wog = (Wout * np.tile(gs, H)[:, None])
    wo = np.ascontiguousarray(
        Wog.reshape(H, 64, OUT_DIM).transpose(1, 0, 2)).astype(bfdt)
    nlam = np.full((128, 1), -lam, np.float32)

    xsT = [np.ascontiguousarray(x[b, 0].T).astype(bfdt) for b in range(B)]
    maskT = [np.ascontiguousarray(mask[b].T.astype(np.float32)).astype(bfdt)
             for b in range(B)]

    if "nc" not in _CACHE:
        _CACHE["nc"] = _build_program()
    nc = _CACHE["nc"]

    in_maps = []
    for core in range(8):
        b, qc = divmod(core, 4)
        in_maps.append({
            "xsT": xsT[b],
            "xqT": np.ascontiguousarray(
                xsT[b][:, qc * QSL:(qc + 1) * QSL]),
            "maskT": np.ascontiguousarray(
                maskT[b][:, qc * QSL:(qc + 1) * QSL]),
            "w": W,
            "wo": wo,
            "nlam": nlam,
        })

    r = run_bass_kernel_spmd(nc, in_maps, list(range(8)))
    _CACHE["last_results"] = r
    res = r.results

    out = np.empty((B, 1, L, OUT_DIM), np.float32)
    for core in range(8):
        b, qc = divmod(core, 4)
        out[b, 0, qc * QSL:(qc + 1) * QSL, :] = res[core]["out"]
    return out



# revision 38
# speedup vs baseline: 1.0118x; 1.0118x over previous
"""DiffAttn3d Trainium2 kernel (v6.1: v6 + single-Newton fast-invsqrt).

8-core sharding: core c -> (batch b = c//4, query slice qs = (c%4)*512).
Each core computes its 512-query slice of the full differential-attention
block (all 16 n-heads) and the final output projection for that slice.

Structure:
- Host pre-transposes x (xsT/xqT bf16) and pre-packs all weights in bf16.
- Heads packed 4-per-128-partitions (offsets 0/32/64/96); the two QK^T
  matmuls of a head pair run concurrently on their two 32-row PE groups
  (explicit tile_position row tiling).
- Software-pipelined by head pair: phase i streams QK -> exp ([128,1024]
  ACT batches, double-buffered 2-bank score PSUM) -> multiplicative bf16
  mask (DVE broadcast AP) into pair i's SBUF e-buffer, while the PE queue
  interleaves the AV matmuls of pair i-1 reading the previous e-buffer.
  Engine queues are strict FIFO; this emission order is the schedule.
- AV accumulates [80, 512] per n-head in PSUM (ones columns 64:80 give
  the softmax denominator; 16 of them so the xbar 16-row tile covers the
  denominator in the same transpose).
- Epilogue per pair: PSUM->SBUF bf16 copies, then ONE 3-D xbar DMA
  transpose per tensor ([80,512] -> [128,4,80]) back to natural [q, d]
  layout, batched DVE combine + fast inverse sqrt on [128,4]-shaped
  scalars (partition-parallel), one DMA transpose of the scaled attn
  rows ([128,4,128]; junk cols never read), and the output projection
  accumulated across all 8 pairs x 4 subtiles in one persistent PSUM
  bank (single accumulation group - start clears has_written bank-wide).
PSUM: 4 (scores x2) + 2 (av) + 1 (out proj) = 7 of 8 banks.

The fast inverse sqrt uses ONE Newton iteration (max ~0.2% error on the
RMS-norm scale, far inside the 2e-2 budget); the second iteration's 4
DVE ops x 8 pairs sat on the busiest engine. Measured 190464 ns vs
192755 ns with two iterations (rel err 5.01e-3 vs 4.91e-3).

Alternatives measured SLOWER than this three-way balance (PE ~138us /
ACT ~138us / DVE ~144us active, ~73% pipeline coupling efficiency):
- Folding the mask additively into score PSUM via identity matmuls
  (PE + ~450ns/unit of serial full-array matmuls -> 204-214us).
- fp8e4 exp/V with DoubleRow AV: DR gave only ~5% over bf16 on the
  streaming side (404ns vs 426ns per unit) while fp8-e alone costs
  ~1.4e-2 rel err. Not worth it.
- Schraudolph bit-trick exp on DVE (tensor_scalar f32->int16, RNE
  saturating, bits = 184.665*s + 16250.9): works and is accurate to
  ~3%/element, but DVE has no spare capacity outside phase 0.
- Routing deferred projections through the idle out-proj PSUM bank:
  cross-engine waits inside the PE FIFO stall everything (241us).
"""

import math
import numpy as np

B, L, IN_DIM, OUT_DIM = 2, 2048, 128, 128
H, DH = 8, 32
ED = H * DH * 2          # 512
NH = 2 * H               # 16 n-heads
DEPTH = 1
LAMBDA_INIT = 0.8 - 0.6 * math.exp(-0.3 * (DEPTH + 1))
EPS = 1e-8

QSL = 512                # queries per core
NKC = L // 128           # 16 key chunks
NQS = QSL // 128         # 4 query subtiles
NCH = 4                  # head chunks: 4 heads per 128 partitions
AVP = 80                 # av rows: 64 v-dims + 16 ones (xbar tile = 16 rows)

_CACHE = {}


def _build_program():
    import concourse.bass as bass
    import concourse.tile as tile
    from concourse import bacc, mybir

    f32 = mybir.dt.float32
    bf16 = mybir.dt.bfloat16
    u32 = mybir.dt.uint32
    AF = mybir.ActivationFunctionType
    ALU = mybir.AluOpType

    nc = bacc.Bacc("TRN2", target_bir_lowering=False, debug=False,
                   num_devices=8)

    xsT_d = nc.declare_dram_parameter("xsT", [IN_DIM, L], bf16, isOutput=False)
    xqT_d = nc.declare_dram_parameter("xqT", [IN_DIM, QSL], bf16,
                                      isOutput=False)
    mT_d = nc.declare_dram_parameter("maskT", [L, QSL], bf16, isOutput=False)
    # w = [Wq4 | Wk4 | Wv]: Wq/Wk packed 4 heads per 128 cols (offsets
    # 0/32/64/96), Wq pre-scaled by DH^-0.5.
    w_d = nc.declare_dram_parameter("w", [IN_DIM, 3 * ED], bf16,
                                    isOutput=False)
    wo_d = nc.declare_dram_parameter("wo", [64, H, OUT_DIM], bf16,
                                     isOutput=False)
    nlam_d = nc.declare_dram_parameter("nlam", [128, 1], f32, isOutput=False)
    out_d = nc.declare_dram_parameter("out", [QSL, OUT_DIM], f32, isOutput=True)

    with tile.TileContext(nc) as tc:
        with (
            tc.tile_pool(name="const", bufs=1) as const,
            tc.tile_pool(name="psA", bufs=2, space=bass.MemorySpace.PSUM) as psA,
            tc.tile_pool(name="avp", bufs=1, space=bass.MemorySpace.PSUM) as avp,
            tc.tile_pool(name="outp", bufs=1, space=bass.MemorySpace.PSUM) as outp,
            tc.tile_pool(name="epool", bufs=2) as epool,
            tc.tile_pool(name="natp", bufs=2) as natp,
            tc.tile_pool(name="tinyp", bufs=4) as tinyp,
            tc.tile_pool(name="tmpp", bufs=3) as tmpp,
        ):
            # ---- constants / weights (all DMA'd pre-packed in bf16) ----
            w_sb = const.tile([128, 3 * ED], bf16)
            nc.sync.dma_start(w_sb[:], w_d[:])
            wo_sb = const.tile([64, H, OUT_DIM], bf16)
            nc.sync.dma_start(wo_sb[:], wo_d[:])
            nlam_sb = const.tile([128, 1], f32)
            nc.sync.dma_start(nlam_sb[:], nlam_d[:])
            xsT = const.tile([128, L], bf16)
            nc.sync.dma_start(xsT[:], xsT_d[:])
            xqT = const.tile([128, QSL], bf16)
            nc.sync.dma_start(xqT[:], xqT_d[:])
            mkf = const.tile([128, NKC, QSL], bf16)
            nc.sync.dma_start(mkf[:], mT_d.rearrange("(kc p) q -> p kc q",
                                                     p=128))
            magic = const.tile([128, NQS], u32)
            nc.vector.memset(magic[:], 0x5F3759DF)

            # ---- projections (bf16 in/out, fp32 psum) ----
            # Only chunk 0's qT/kT are emitted up front (the minimum to
            # start the exp stream); everything else is deferred into the
            # early phases' kc slots via setup_items.
            qT = const.tile([128, NCH, QSL], bf16)
            kT = const.tile([128, NCH, L], bf16)
            vp = const.tile([128, NKC, H, AVP], bf16)
            nc.vector.memset(vp[:, :, :, 64:AVP], 1.0)

            def proj_q(c):
                def emit():
                    ps = psA.tile([128, 2, QSL], f32, tag="sps", name="ps")
                    nc.tensor.matmul(ps[:, 0, :],
                                     w_sb[:, c * 128:(c + 1) * 128],
                                     xqT[:], start=True, stop=True)
                    nc.vector.tensor_copy(qT[:, c, :], ps[:, 0, :])
                return emit

            def proj_k(c, h2):
                def emit():
                    ps = psA.tile([128, 2, QSL], f32, tag="sps", name="ps")
                    for s in range(2):
                        nc.tensor.matmul(
                            ps[:, s, :],
                            w_sb[:, ED + c * 128:ED + (c + 1) * 128],
                            xsT[:, (h2 * 2 + s) * 512:(h2 * 2 + s + 1) * 512],
                            start=True, stop=True)
                    nc.vector.tensor_copy(
                        kT[:, c, h2 * 1024:(h2 + 1) * 1024],
                        ps[:].rearrange("p s q -> p (s q)"))
                return emit

            def proj_v(st2):
                def emit():
                    ps = psA.tile([128, 2, QSL], f32, tag="sps", name="ps")
                    for s in range(2):
                        st = st2 * 2 + s
                        nc.tensor.matmul(ps[:, s, :],
                                         xsT[:, st * 128:(st + 1) * 128],
                                         w_sb[:, 2 * ED:3 * ED],
                                         start=True, stop=True)
                    for s in range(2):
                        st = st2 * 2 + s
                        nc.vector.tensor_copy(
                            vp[:, st, :, 0:64],
                            ps[:, s, :].rearrange("p (h d) -> p h d", h=H))
                return emit

            for f in (proj_q(0), proj_k(0, 0), proj_k(0, 1)):
                f()
            # deferred: vp during phase 0 (used by AV from phase 1 on),
            # chunk c's qT/kT during phase 2c-2 (used from phase 2c on)
            setup_items = {
                0: [proj_v(s) for s in range(NKC // 2)],
                1: [proj_q(1), proj_k(1, 0), proj_k(1, 1)],
                2: [proj_q(2), proj_k(2, 0), proj_k(2, 1)],
                3: [proj_q(3), proj_k(3, 0), proj_k(3, 1)],
            }

            out_ps = outp.tile([128, NQS, 128], f32, tag="op", name="out_ps")
            n_outmm = [0]

            def pass1(i, kc, eb):
                c, p = divmod(i, 2)
                sps = psA.tile([128, 2, QSL], f32, tag="sps", name="sps")
                for j in range(2):
                    r = 64 * p + 32 * j
                    nc.tensor.matmul(
                        sps[:, j, :],
                        kT[r:r + 32, c, kc * 128:(kc + 1) * 128],
                        qT[r:r + 32, c, :],
                        start=True, stop=True, tile_position=(r, 0))
                nc.scalar.activation(eb[:, kc, :, :], sps[:], AF.Exp)
                # NOTE: batching this mask multiply over multiple key
                # chunks (4D AP with a broadcast middle dim) mis-lowers on
                # DVE (rel err 0.37) - keep it per-chunk.
                mb = mkf[:, kc, :].unsqueeze(1).broadcast_to([128, 2, QSL])
                nc.vector.tensor_tensor(eb[:, kc, :, :], eb[:, kc, :, :],
                                        mb, ALU.mult)

            def av_step(i, kc, eb, av):
                for j in range(2):
                    nc.tensor.matmul(av[j][:], vp[:, kc, i, :],
                                     eb[:, kc, j, :],
                                     start=(kc == 0), stop=(kc == NKC - 1))

            def epilogue(i, av):
                nat = []
                for j in range(2):
                    a_sb = tmpp.tile([AVP, QSL], bf16, tag=f"a_sb{j}")
                    nc.vector.tensor_copy(a_sb[:], av[j][:])
                    nt = natp.tile([128, NQS, AVP], bf16, tag=f"nat{j}",
                                   name=f"nat{j}")
                    nc.sync.dma_start_transpose(nt[:], a_sb[:])
                    nat.append(nt)

                r0v = tinyp.tile([128, NQS, 1], f32, tag="r0v")
                nc.vector.reciprocal(r0v[:], nat[0][:, :, 64:65])
                r1v = tinyp.tile([128, NQS, 1], f32, tag="r1v")
                nc.vector.reciprocal(r1v[:], nat[1][:, :, 64:65])
                r1p = tinyp.tile([128, NQS, 1], f32, tag="r1p")
                nc.vector.tensor_scalar(r1p[:], r1v[:], nlam_sb[:], None,
                                        ALU.mult)

                t0 = tmpp.tile([128, NQS, 64], f32, tag="t0")
                nc.vector.tensor_tensor(
                    t0[:], nat[0][:, :, 0:64],
                    r0v[:].broadcast_to([128, NQS, 64]), ALU.mult)
                t1 = tmpp.tile([128, NQS, 64], f32, tag="t1")
                nc.vector.tensor_tensor(
                    t1[:], nat[1][:, :, 0:64],
                    r1p[:].broadcast_to([128, NQS, 64]), ALU.mult)
                at4 = tmpp.tile([128, NQS, 64], bf16, tag="at4")
                nc.vector.tensor_tensor(at4[:], t0[:], t1[:], ALU.add)
                sq4 = tmpp.tile([128, NQS, 64], f32, tag="sq4")
                nc.vector.tensor_tensor(sq4[:], at4[:], at4[:], ALU.mult)
                ss4 = tinyp.tile([128, NQS], f32, tag="ss4")
                nc.vector.tensor_reduce(ss4[:], sq4[:],
                                        mybir.AxisListType.X, ALU.add)

                # rr4 = 1/sqrt(ss4/64): fast inverse sqrt on DVE
                msx = tinyp.tile([128, NQS], f32, tag="msx")
                nc.vector.tensor_scalar(msx[:], ss4[:], 1.0 / 64, None,
                                        ALU.mult)
                sh = tinyp.tile([128, NQS], u32, tag="sh")
                nc.vector.tensor_scalar(sh[:], msx[:].bitcast(u32), 1,
                                        None, ALU.logical_shift_right)
                rr4 = tinyp.tile([128, NQS], f32, tag="rr4")
                nc.vector.tensor_tensor(rr4[:].bitcast(u32), magic[:],
                                        sh[:], ALU.subtract)
                nwu = tinyp.tile([128, NQS], f32, tag="nwu")
                nww = tinyp.tile([128, NQS], f32, tag="nww")
                for _ in range(1):
                    nc.vector.tensor_tensor(nwu[:], rr4[:], rr4[:], ALU.mult)
                    nc.vector.scalar_tensor_tensor(
                        nwu[:], nwu[:], 0.5, msx[:], ALU.mult, ALU.mult)
                    nc.vector.tensor_scalar(nww[:], nwu[:], -1.0, 1.5,
                                            ALU.mult, ALU.add)
                    nc.vector.tensor_tensor(rr4[:], rr4[:], nww[:], ALU.mult)

                # at_s cols 64:128 are junk; the transposed junk rows are
                # never read (projection lhsT slice [0:64])
                at_s = tmpp.tile([128, NQS, 128], bf16, tag="at_s")
                nc.vector.tensor_tensor(
                    at_s[:, :, 0:64], at4[:],
                    rr4[:].unsqueeze(2).broadcast_to([128, NQS, 64]),
                    ALU.mult)
                atT = natp.tile([128, NQS, 128], bf16, tag="atT",
                                name=f"atT{i}")
                nc.sync.dma_start_transpose(atT[:], at_s[:])
                for q in range(NQS):
                    # single accumulation group for the whole bank: start
                    # clears has_written bank-wide
                    nc.tensor.matmul(out_ps[:, q, :], atT[0:64, q, :],
                                     wo_sb[:, i, :],
                                     start=(n_outmm[0] == 0),
                                     stop=(n_outmm[0] == H * NQS - 1))
                    n_outmm[0] += 1

            # ---- attention: software-pipelined over 8 pairs ----
            # phase i: pass1(i) + AV(i-1); the last pair's AV runs inline
            # in phase 7 (after its own mask) so there is no drain phase.
            ebufs, avbufs = {}, {}
            for i in range(H + 1):
                if i < H:
                    ebufs[i] = epool.tile([128, NKC, 2, QSL], bf16,
                                          tag="eall", name=f"eall{i}")
                    avbufs[i] = [avp.tile([AVP, QSL], f32, tag=f"av{j}",
                                          name=f"av{j}_{i}")
                                 for j in range(2)]
                items = setup_items.get(i, [])
                for kc in range(NKC):
                    if i < H:
                        pass1(i, kc, ebufs[i])
                    if items and kc % 2 == 0 and kc // 2 < len(items):
                        items[kc // 2]()
                    if i >= 1:
                        av_step(i - 1, kc, ebufs[i - 1], avbufs[i - 1])
                if i >= 1:
                    epilogue(i - 1, avbufs[i - 1])
                    del ebufs[i - 1], avbufs[i - 1]

            out_sb = const.tile([128, NQS, 128], f32)
            nc.vector.tensor_copy(out_sb[:], out_ps[:])
            nc.sync.dma_start(out_d.rearrange("(s p) o -> p s o", p=128),
                              out_sb[:])

    nc.compile()
    return nc


def kernel(**inputs):
    import ml_dtypes
    from concourse.bass_utils import run_bass_kernel_spmd

    bfdt = ml_dtypes.bfloat16

    x = np.asarray(inputs["x"], np.float32)
    mask = np.asarray(inputs["mask_2d"])
    Wq = np.asarray(inputs["Wq"], np.float32)
    Wkv = np.asarray(inputs["Wkv"], np.float32)
    Wout = np.asarray(inputs["Wout"], np.float32)
    lq1 = np.asarray(inputs["lambda_q1"], np.float32)
    lk1 = np.asarray(inputs["lambda_k1"], np.float32)
    lq2 = np.asarray(inputs["lambda_q2"], np.float32)
    lk2 = np.asarray(inputs["lambda_k2"], np.float32)
    gamma = np.asarray(inputs["gamma"], np.float32)

    lam = float(np.exp(np.sum(lq1 * lk1)) - np.exp(np.sum(lq2 * lk2))
                + LAMBDA_INIT)
    Wq_s = (Wq * DH ** -0.5).astype(np.float32)
    Wk = Wkv[:, :ED]
    Wv = Wkv[:, ED:]

    def pack_heads4(Wm):
        # chunk c (128 cols) holds heads 4c..4c+3 at col offsets 0/32/64/96
        out = np.empty((IN_DIM, NCH * 128), np.float32)
        for n in range(NH):
            c, r = divmod(n, 4)
            out[:, c * 128 + r * 32:c * 128 + r * 32 + 32] = \
                Wm[:, n * DH:(n + 1) * DH]
        return out

    W = np.concatenate([pack_heads4(Wq_s), pack_heads4(Wk), Wv],
                       axis=1).astype(bfdt)
    gs = (gamma * (1.0 - LAMBDA_INIT)).astype(np.float32)
    Wog = (Wout * np.tile(gs, H)[:, None])
    wo = np.ascontiguousarray(
        Wog.reshape(H, 64, OUT_DIM).transpose(1, 0, 2)).astype(bfdt)
    nlam = np.full((128, 1), -lam, np.float32)

    xsT = [np.ascontiguousarray(x[b, 0].T).astype(bfdt) for b in range(B)]
    maskT = [np.ascontiguousarray(mask[b].T.astype(np.float32)).astype(bfdt)
             for b in range(B)]

    if "nc" not in _CACHE:
        _CACHE["nc"] = _build_program()
    nc = _CACHE["nc"]

    in_maps = []
    for core in range(8):
        b, qc = divmod(core, 4)
        in_maps.append({
            "xsT": xsT[b],
            "xqT": np.ascontiguousarray(
                xsT[b][:, qc * QSL:(qc + 1) * QSL]),
            "maskT": np.ascontiguousarray(
                maskT[b][:, qc * QSL:(qc + 1) * QSL]),
            "w": W,
            "wo": wo,
            "nlam": nlam,
        })

    r = run_bass_kernel_spmd(nc, in_maps, list(range(8)))
    _CACHE["last_results"] = r
    res = r.results

    out = np.empty((B, 1, L, OUT_DIM), np.float32)
    for core in range(8):
        b, qc = divmod(core, 4)
        out[b, 0, qc * QSL:(qc + 1) * QSL, :] = res[core]["out"]
    return out

